# revision 1
# baseline (speedup 1.0000x reference)
"""Trainium2 Bass kernel for nn_Agent_57732950393167 (ragged_sequence).

Strategy
--------
Data-parallel over batches: 32 batches / 8 cores = 4 batches ("groups" g)
per core, each with V=8 vehicles -> 32 vehicles/core.

Key restructurings (vs. the reference):
 * nde = ndf @ W_ns ([T,N,384], 402MB) is NEVER materialized.  It is rank-8
   in the feature dim, so its three uses are folded:
     - K-part: compat_dyn[t,h,n] = sum_f ndf[t,n,f] * qw[t][f,h],
       qw = (Q/4) . W_nsK head-blocks  (an [8,8] matrix per vehicle)
     - V-part: heads_dyn = (sum_n attn*ndf) @ W_nsV  (attn-weighted feature
       sums AF[t,h,f], an [8,8] per vehicle)
     - L-part: logits_dyn[t,n] = sum_f ndf[t,n,f] * (W_nsL . final_Q[t])
 * Big matmuls batch the 64 (vehicle,head) rows of a batch-pair into one
   PE pass using block-diagonal stationary matrices (built on device via a
   replication matmul + select-mask; PE partition bases stay 32-aligned).
 * Transposed layouts (K^T, logitK^T, ndf^T, ndf-natural) are produced on
   the host during input sharding so every device DMA is a large
   contiguous transfer; consts/weights are packed into single tensors and
   loads are spread across the SP/ACT HWDGE rings + SWDGE.
 * Program is phase-ordered (all compat/softmax -> all heads -> all
   logits -> one batched epilogue) to avoid ACT function-table thrash.
 * Softmax runs unnormalized (no max shift -- |scores| < ~15); the 1/sum
   is folded into the single heads-PSUM rescale via a [128,1] per-pair
   reciprocal vector.
 * Epilogue: per-group logits (10*tanh and mask terms) are accumulated
   straight into one [32,1024] tile by selector matmuls, one
   max_with_indices / one Exp; lp = M - ln(S), prob = 1/(S*e^-M) via DVE
   reciprocal (one Exp + one Ln table load total).

log(mask) is approximated by MASK_BIG*(mask-1) with MASK_BIG=50 (exact to
~1e-9 relative in the final softmax sums).
"""

import numpy as np

B, N, D, H, V = 32, 1024, 128, 8, 8
KS = D // H            # 16
F_V = 4
F_ND = 8
TANH_CLIP = 10.0
MASK_BIG = 50.0
NCORES = 8
G = B // NCORES        # 4 groups (batches) per core
NPAIR = G // 2         # 2 batch-pairs per core

_PROGRAM_CACHE = {}


def _build_const_pack():
    """[128, 977] f32: all device constants, one DMA."""
    KSl = KS
    cp = np.zeros((128, 977), dtype=np.float32)
    cp[:, 0:128] = np.eye(128, dtype=np.float32)                  # ident
    cp[:, 128:192] = np.tile(np.eye(64, dtype=np.float32), (2, 1))  # identpad
    cp[0:F_ND, 192:320] = np.tile(np.eye(F_ND, dtype=np.float32),
                                  (1, 16))                        # repl
    dm = np.zeros((128, 128), dtype=np.float32)                   # diagmask
    for p in range(128):
        g2, hv = divmod(p, 64)
        a = g2 * 8 + (hv % 8)
        dm[p, a * 8:(a + 1) * 8] = 1.0
    cp[:, 320:448] = dm
    bd = np.zeros((128, 128), dtype=np.float32)                   # bdsel
    for p in range(128):
        a = p // 8
        g2, v = divmod(a, 8)
        bd[p, g2 * 64 + v:g2 * 64 + 64:8] = 1.0
    cp[:, 448:576] = bd
    fw = np.zeros((128, 16), dtype=np.float32)                    # fwsel
    for p in range(128):
        fw[p, p // 8] = 1.0
    cp[:, 576:592] = fw
    hs = np.zeros((128, 64), dtype=np.float32)                    # hsel
    for hk in range(128):
        h = hk // KSl
        hs[hk, h * 8:(h + 1) * 8] = 1.0
    cp[:, 592:656] = hs
    hb = np.zeros((128, 128), dtype=np.float32)                   # hselb
    for d in range(128):
        h = d // KSl
        for g2 in range(2):
            hb[d, g2 * 64 + h * V:g2 * 64 + (h + 1) * V] = 1.0
    cp[:, 656:784] = hb
    rb = np.zeros((V, H * V), dtype=np.float32)                   # replbig
    for v in range(V):
        rb[v, v::V] = 1.0
    cp[0:V, 784:848] = rb
    sc = np.zeros((V, G * 32), dtype=np.float32)                  # selc
    for g in range(G):
        for v in range(V):
            sc[v, g * 32 + g * 8 + v] = 1.0
    cp[0:V, 848:976] = sc
    cp[0:32, 976] = 8192.0 - 1024.0 * (np.arange(32) % 8)         # c8k32
    return cp


# --------------------------------------------------------------------------
# Device program
# --------------------------------------------------------------------------

def _build_program():
    import contextlib

    import concourse.bacc as bacc
    import concourse.tile as tile
    import concourse.mybir as mybir

    dt = mybir.dt
    f32 = dt.float32
    f32r = dt.float32r
    AF_EXP = mybir.ActivationFunctionType.Exp
    AF_TANH = mybir.ActivationFunctionType.Tanh
    AF_LN = mybir.ActivationFunctionType.Ln
    AF_IDENT = mybir.ActivationFunctionType.Identity
    AF_COPY = mybir.ActivationFunctionType.Copy
    OP = mybir.AluOpType
    AX = mybir.AxisListType

    nc = bacc.Bacc("TRN2", target_bir_lowering=False, debug=False,
                   num_devices=NCORES)

    # ---- external inputs (per-core shards, host-prepped layouts) ----
    kt_in = nc.dram_tensor("kt_in", [G, 128, N], f32r, kind="ExternalInput")
    lt_in = nc.dram_tensor("lt_in", [G, 128, N], f32r, kind="ExternalInput")
    vn_in = nc.dram_tensor("vn_in", [G, N, 128], f32r, kind="ExternalInput")
    ndftm_in = nc.dram_tensor("ndftm_in", [G, 72, N], f32r,
                              kind="ExternalInput")
    mbs_in = nc.dram_tensor("mbs_in", [V, G * N], f32r,
                            kind="ExternalInput")
    ndfn_in = nc.dram_tensor("ndfn_in", [NPAIR, N, 128], f32r,
                             kind="ExternalInput")
    wpack_in = nc.dram_tensor("wpack_in", [128, 548], f32,
                              kind="ExternalInput")
    wpackr_in = nc.dram_tensor("wpackr_in", [128, 16], f32r,
                               kind="ExternalInput")

    res_out = nc.dram_tensor("res_out", [G, 4], f32, kind="ExternalOutput")

    cpack_c = nc.inline_tensor(_build_const_pack(), name="cpack_c")

    with tile.TileContext(nc) as tc:
        with contextlib.ExitStack() as ctx:
            sb = ctx.enter_context(tc.tile_pool(name="sb", bufs=1))
            scr = ctx.enter_context(tc.tile_pool(name="scr", bufs=4))
            acc = ctx.enter_context(
                tc.tile_pool(name="acc", bufs=2, space="PSUM"))
            lgp = ctx.enter_context(
                tc.tile_pool(name="lgp", bufs=1, space="PSUM"))
            tp = ctx.enter_context(
                tc.tile_pool(name="tp", bufs=2, space="PSUM"))

            def P(name, shape, dtype=f32):
                return sb.tile(shape, dtype, name=name, tag=name)

            def S(name, shape, dtype=f32):
                if shape[-1] >= 512:
                    return scr.tile(shape, dtype, name=name, tag="sbig",
                                    bufs=4)
                return scr.tile(shape, dtype, name=name, tag="ssml", bufs=8)

            # ================= persistent SBUF tiles =================
            kt = P("kt", [128, G * N], f32r)       # K^T  (g,n) cols
            lt = P("lt", [128, G * N], f32r)       # logitK^T
            ndftm = [P(f"ndftm{g}", [72, N], f32r) for g in range(G)]
            mbs = P("mbs", [V, G * N], f32r)
            rhsha = [P(f"rhsha{p}", [128, 3 * N], f32r) for p in range(NPAIR)]
            attnt = [P(f"attntp{p}", [128, N]) for p in range(NPAIR)]
            attnnt = [P(f"attnnt{p}", [128, 8 * 128], f32r)
                      for p in range(NPAIR)]
            cpack = P("cpack", [128, 977])
            wpack = P("wpack", [128, 548])
            wpackr = P("wpackr", [128, 16], f32r)
            replbig_r = P("replbig_r", [V, H * V], f32r)
            selc_r = P("selc_r", [V, G * 32], f32r)
            fctq = P("fctq", [128, G])
            queryt = P("queryt", [128, G * V])     # 0.25-scaled query^T
            blockq = [P(f"blockq{p}", [128, 128], f32r) for p in range(NPAIR)]
            bdq72 = [P(f"bdq72_{g}", [72, 64], f32r) for g in range(G)]
            ha_sb = [P(f"hasb{p}", [128, 384]) for p in range(NPAIR)]
            afdt = [P(f"afdt{p}", [F_ND, 128]) for p in range(NPAIR)]
            hct = [P(f"hctp{p}", [128, 2 * V]) for p in range(NPAIR)]
            fqt = [P(f"fqt{p}", [128, 2 * V], f32r) for p in range(NPAIR)]
            bdfw = [P(f"bdfw{g}_g", [64, V], f32r) for g in range(G)]
            lgf32 = P("lgf32", [32, N])
            rinv_p = [P(f"rinvp{p}", [128, 1]) for p in range(NPAIR)]

            # PE warm-up: ~4us of back-to-back matmuls on a zeroed tile
            # so the HAM un-throttles the PE clock before real work
            prime_sb = P("prime_sb", [128, 512])
            nc.vector.memset(prime_sb[:], 0.0)
            for i in range(4):
                prime_ps = tp.tile([128, 512], f32, name=f"prime{i}",
                                   tag="tp")
                nc.tensor.matmul(prime_ps[:], prime_sb[:, 0:128],
                                 prime_sb[:], start=True, stop=True,
                                 skip_group_check=True)

            # const/weight slices
            ident = cpack[:, 0:128]
            identpad = cpack[:, 128:192]
            repl = cpack[0:F_ND, 192:320]
            diagmask = cpack[:, 320:448]
            bdsel = cpack[:, 448:576]
            fwsel = cpack[:, 576:592]
            hsel = cpack[:, 592:656]
            hselb = cpack[:, 656:784]
            replbig = cpack[0:V, 784:848]
            selc = cpack[0:V, 848:976]
            c8k32 = cpack[0:32, 976:977]
            wcs_hi = wpack[:, 0:128]
            wout = wpack[:, 128:256]
            wnsv = wpack[0:F_ND, 256:384]
            wcs_lo = wpack[0:F_V, 384:512]
            fct = wpack[:, 512:516]
            vdft = wpack[0:F_V, 516:548]
            wnskt = wpackr[:, 0:F_ND]
            wnslt = wpackr[:, F_ND:2 * F_ND]

            # ================= loads =================
            # HWDGE issues cost the issuing engine ~0.7us each; sync is
            # otherwise idle so it carries most.  cpack rides the ACT ring
            # (its only consumer waits on it anyway); lt (needed late)
            # rides SWDGE with rhsha.
            nc.scalar.dma_start(cpack[:], cpack_c.ap())
            nc.sync.dma_start(wpack[:], wpack_in.ap())
            nc.sync.dma_start(wpackr[:], wpackr_in.ap())
            nc.sync.dma_start(ndftm[0][:], ndftm_in[0])
            nc.sync.dma_start(kt[:, 0:N], kt_in[0])
            nc.sync.dma_start(ndftm[1][:], ndftm_in[1])
            nc.sync.dma_start(kt[:, N:2 * N], kt_in[1])
            nc.sync.dma_start(ndftm[2][:], ndftm_in[2])
            nc.sync.dma_start(ndftm[3][:], ndftm_in[3])
            nc.sync.dma_start(mbs[:], mbs_in.ap())
            nc.sync.dma_start(kt[:, 2 * N:3 * N], kt_in[2])
            nc.sync.dma_start(kt[:, 3 * N:4 * N], kt_in[3])

            # gpsimd (SWDGE): rhsha contiguous regions
            for p in range(NPAIR):
                for g2 in range(2):
                    nc.gpsimd.dma_start(
                        rhsha[p].rearrange("q (c w) -> q c w", w=128)
                        [:, g2 * 8:(g2 + 1) * 8, :],
                        vn_in[2 * p + g2].rearrange("(c q) w -> q c w",
                                                    q=128))
                nc.gpsimd.dma_start(
                    rhsha[p].rearrange("q (c w) -> q c w", w=128)
                    [:, 16:24, :],
                    ndfn_in[p].rearrange("(c q) w -> q c w", q=128))
            for g in range(G):
                nc.gpsimd.dma_start(lt[:, g * N:(g + 1) * N], lt_in[g])

            # fctq = 0.25*fc^T
            nc.vector.tensor_scalar_mul(fctq[:], fct, 0.25)
            nc.vector.tensor_copy(replbig_r[:], replbig)
            nc.vector.tensor_copy(selc_r[:], selc)

            # ================= phase A: query / qw smalls =================
            fct8a = S("fct8a", [128, G * V])
            for g in range(G):
                nc.scalar.activation(fct8a[:, g * V:(g + 1) * V],
                                     ident[:, 0:V], AF_IDENT,
                                     bias=fct[:, g:g + 1], scale=0.0)
            qt_ps = tp.tile([128, G * V], f32, name="qt_ps", tag="tp")
            nc.tensor.matmul(qt_ps[:], wcs_hi, fct8a[:],
                             start=True, stop=False, skip_group_check=True)
            nc.tensor.matmul(qt_ps[:], wcs_lo, vdft,
                             start=False, stop=True, skip_group_check=True)
            for g in range(G):
                # queryt = 0.25*(cur + fc)
                nc.scalar.activation(queryt[:, g * V:(g + 1) * V],
                                     qt_ps[:, g * V:(g + 1) * V],
                                     AF_IDENT, bias=fctq[:, g:g + 1],
                                     scale=0.25)

            for p in range(NPAIR):
                # blockq[d, (g2,h,v)] = queryt[d, (g,v)] * (h == d//16)
                qview = (queryt[:, 2 * p * V:(2 * p + 2) * V]
                         .rearrange("d (g2 v) -> d g2 v", g2=2)
                         .unsqueeze(2).broadcast_to([128, 2, H, V]))
                nc.vector.tensor_tensor(
                    blockq[p].rearrange("d (g2 h v) -> d g2 h v", g2=2, h=H),
                    qview, hselb.rearrange("d (g2 h v) -> d g2 h v",
                                           g2=2, h=H),
                    OP.mult)
                # qw_all[f, (g2,h,v)] then replicate+mask into block-diag
                qw_ps = tp.tile([F_ND, 128], f32, name=f"qw_ps{p}", tag="tp")
                nc.tensor.matmul(qw_ps[:], wnskt, blockq[p][:],
                                 start=True, stop=True)
                qw_sbt = S(f"qw_sbt{p}", [F_ND, 128])
                nc.vector.tensor_copy(qw_sbt[:], qw_ps[:])
                qwr_ps = tp.tile([128, 128], f32, name=f"qwr_ps{p}", tag="tp")
                nc.tensor.matmul(qwr_ps[:], repl, qw_sbt[:],
                                 start=True, stop=True)
                for g2 in range(2):
                    g = 2 * p + g2
                    gsl = slice(g2 * 64, (g2 + 1) * 64)
                    nc.vector.tensor_tensor(bdq72[g][0:64, :],
                                            qwr_ps[gsl, gsl],
                                            bdsel[gsl, gsl], OP.mult)
                    nc.vector.tensor_copy(bdq72[g][64:72, :], replbig)

            # ============ phase C: compat + softmax, all groups ==========
            for p in range(NPAIR):
                for g2 in range(2):
                    g = 2 * p + g2
                    gsl = slice(g2 * 64, (g2 + 1) * 64)
                    compat = acc.tile([64, N], f32, name=f"compat{g}",
                                      tag="acc")
                    # dyn + mask in one 72-row contraction; static after
                    for half in range(2):
                        sl = slice(half * 512, (half + 1) * 512)
                        nc.tensor.matmul(
                            compat[:, sl], bdq72[g][:],
                            ndftm[g][:, sl],
                            start=True, stop=False, skip_group_check=True)
                    for half in range(2):
                        sl = slice(half * 512, (half + 1) * 512)
                        nc.tensor.matmul(
                            compat[:, sl],
                            blockq[p][:, gsl],
                            kt[:, g * N:(g + 1) * N][:, sl],
                            start=False, stop=True, skip_group_check=True)

                    # unnormalized exp into the pair tile (|compat| < ~15)
                    rsum = S(f"rsum{g}", [64, 1])
                    nc.scalar.activation(attnt[p][gsl, :], compat[:],
                                         AF_EXP, accum_out=rsum[:])
                    nc.vector.reciprocal(rinv_p[p][gsl, :], rsum[:])

                # attn^T -> attn_n: [128,128] transposes, paired copies
                for c2 in range(4):
                    at_ps = tp.tile([128, 256], f32,
                                    name=f"at_ps{p}_{c2}", tag="tp")
                    for j in range(2):
                        c = 2 * c2 + j
                        nc.tensor.matmul(
                            at_ps[:, j * 128:(j + 1) * 128],
                            attnt[p][:, c * 128:(c + 1) * 128],
                            ident,
                            is_transpose=True,
                            start=True, stop=True,
                            skip_group_check=True)
                    dst = (attnnt[p]
                           .rearrange("q (c w) -> q c w", w=128)
                           [:, 2 * c2:2 * c2 + 2, :])
                    src_ap = at_ps.rearrange("q (j w) -> q j w", j=2)
                    if c2 % 2 == 0:
                        nc.scalar.activation(dst, src_ap, AF_COPY)
                    else:
                        nc.vector.tensor_copy(dst, src_ap)

            # ============ phase D: heads + AF + final_Q, all pairs =======
            for p in range(NPAIR):
                # heads+AF over the 3 contiguous rhsha regions (2-dim
                # free AP); normalization via rinv in the PSUM->SBUF move
                ha_ps = tp.tile([128, 384], f32, name=f"ha_ps{p}", tag="tp")
                rh = rhsha[p].rearrange("q (r w) -> q r w", r=3)
                for c in range(8):
                    nc.tensor.matmul(ha_ps[:],
                                     attnnt[p][:, c * 128:(c + 1) * 128],
                                     rh[:, :, c * 128:(c + 1) * 128],
                                     start=(c == 0), stop=(c == 7))
                nc.vector.tensor_scalar_mul(ha_sb[p][:], ha_ps[:],
                                            rinv_p[p][:])

                # AF diag-extract -> AFd [128, F] -> AFd^T
                aftmp = S(f"aftmp{p}", [128, 128])
                nc.vector.tensor_tensor(aftmp[:], ha_sb[p][:, 256:384],
                                        diagmask, OP.mult)
                afd = S(f"afd{p}", [128, F_ND])
                nc.vector.tensor_reduce(
                    afd[:], aftmp.rearrange("q (a f) -> q f a", f=F_ND),
                    AX.X, OP.add)
                afd_ps = tp.tile([F_ND, 128], f32, name=f"afd_ps{p}",
                                 tag="tp")
                nc.tensor.matmul(afd_ps[:], afd[:], ident,
                                 is_transpose=True, start=True, stop=True)
                nc.vector.tensor_copy(afdt[p][:], afd_ps[:])

                # heads -> hcT -> final_Q^T per group
                fqp = tp.tile([128, 2 * V], f32, name=f"fqp{p}", tag="tp")
                for g2 in range(2):
                    g = 2 * p + g2
                    hq_ps = tp.tile([128, 64], f32, name=f"hq_ps{g}",
                                    tag="tp")
                    nc.tensor.matmul(
                        hq_ps[:],
                        ha_sb[p][g2 * 64:(g2 + 1) * 64,
                                 g2 * 128:(g2 + 1) * 128],
                        identpad[g2 * 64:(g2 + 1) * 64, :],
                        is_transpose=True, start=True, stop=False,
                        skip_group_check=True)
                    nc.tensor.matmul(
                        hq_ps[:], wnsv,
                        afdt[p][:, g2 * 64:(g2 + 1) * 64],
                        start=False, stop=True, skip_group_check=True)
                    hqs = S(f"hqs{g}", [128, 64])
                    nc.vector.tensor_tensor(hqs[:], hq_ps[:], hsel,
                                            OP.mult)
                    nc.vector.tensor_reduce(
                        hct[p][:, g2 * V:(g2 + 1) * V],
                        hqs.rearrange("q (hh v) -> q v hh", v=V),
                        AX.X, OP.add)
                nc.tensor.matmul(fqp[:], wout, hct[p][:],
                                 start=True, stop=True)
                nc.vector.tensor_copy(fqt[p][:], fqp[:])

                # block-diag fw
                fw_ps = tp.tile([F_ND, 2 * V], f32, name=f"fw_ps{p}",
                                tag="tp")
                nc.tensor.matmul(fw_ps[:], wnslt, fqt[p][:],
                                 start=True, stop=True)
                fw_sbt = S(f"fw_sbt{p}", [F_ND, 2 * V])
                nc.vector.tensor_copy(fw_sbt[:], fw_ps[:])
                fwr_ps = tp.tile([128, 2 * V], f32, name=f"fwr_ps{p}",
                                 tag="tp")
                nc.tensor.matmul(fwr_ps[:], repl, fw_sbt[:],
                                 start=True, stop=True)
                for g2 in range(2):
                    g = 2 * p + g2
                    gsl = slice(g2 * 64, (g2 + 1) * 64)
                    vsl = slice(g2 * V, (g2 + 1) * V)
                    nc.vector.tensor_tensor(bdfw[g][:], fwr_ps[gsl, vsl],
                                            fwsel[gsl, vsl], OP.mult)

            # ======== phase E: logits + tanh; stack into [32, N] =========
            lgs_ps = acc.tile([32, N], f32, name="lgs_ps", tag="acc")
            for p in range(NPAIR):
                for g2 in range(2):
                    g = 2 * p + g2
                    gsl = slice(g2 * 64, (g2 + 1) * 64)
                    lg_ps = lgp.tile([V, N], f32, name=f"lg_ps{g}", tag="lg")
                    for half in range(2):
                        sl = slice(half * 512, (half + 1) * 512)
                        nc.tensor.matmul(
                            lg_ps[:, sl],
                            bdfw[g][:],
                            ndftm[g][0:64, sl],
                            start=True, stop=False, skip_group_check=True)
                    for half in range(2):
                        sl = slice(half * 512, (half + 1) * 512)
                        nc.tensor.matmul(
                            lg_ps[:, sl],
                            fqt[p][:, g2 * V:(g2 + 1) * V],
                            lt[:, g * N:(g + 1) * N][:, sl],
                            start=False, stop=True, skip_group_check=True)
                    th = S(f"th{g}", [V, N])
                    nc.scalar.activation(th[:], lg_ps[:], AF_TANH,
                                         scale=float(1.0 / np.sqrt(D)))
                    lgfg = P(f"lgf_{g}", [V, N], f32r)
                    nc.vector.scalar_tensor_tensor(
                        lgfg[:], th[:], TANH_CLIP,
                        mbs[:, g * N:(g + 1) * N],
                        op0=OP.mult, op1=OP.add)
                    # accumulate into the stacked [32, N] tile
                    for half in range(2):
                        sl = slice(half * 512, (half + 1) * 512)
                        nc.tensor.matmul(
                            lgs_ps[:, sl],
                            selc_r[:, g * 32:(g + 1) * 32],
                            lgfg[:, sl],
                            start=(g == 0), stop=(g == G - 1),
                            skip_group_check=True)

            # ============ epilogue: batched flat log-softmax/argmax ======
            nc.vector.tensor_copy(lgf32[:], lgs_ps[:])
            rs32 = S("rs32e", [32, 1])
            expf = S("expfe", [32, N])
            nc.scalar.activation(expf[:], lgs_ps[:], AF_EXP,
                                 accum_out=rs32[:])
            mx8 = S("mx8e", [32, 8])
            ix8 = S("ix8e", [32, 8], dt.uint32)
            nc.vector.max_with_indices(mx8[:], ix8[:], lgf32[:])
            idxf = S("idxfe", [32, 1])
            nc.vector.tensor_copy(idxf[:], ix8[:, 0:1])
            cand = S("cande", [32, 1])
            nc.vector.tensor_tensor(cand[:], c8k32, idxf[:], OP.subtract)

            rmt_ps = tp.tile([1, 32], f32, name="rmt_ps", tag="tp")
            nc.tensor.matmul(rmt_ps[:], mx8[:, 0:1], ident[0:32, 0:32],
                             is_transpose=True, start=True, stop=True)
            rmt = S("rmte", [1, 32])
            nc.vector.tensor_copy(rmt[:], rmt_ps[:])
            rst_ps = tp.tile([1, 32], f32, name="rst_ps", tag="tp")
            nc.tensor.matmul(rst_ps[:], rs32[:], ident[0:32, 0:32],
                             is_transpose=True, start=True, stop=True)
            rst = S("rste", [1, 32])
            nc.vector.tensor_copy(rst[:], rst_ps[:])
            cdt_ps = tp.tile([1, 32], f32, name="cdt_ps", tag="tp")
            nc.tensor.matmul(cdt_ps[:], cand[:], ident[0:32, 0:32],
                             is_transpose=True, start=True, stop=True)
            cdt = S("cdte", [1, 32])
            nc.vector.tensor_copy(cdt[:], cdt_ps[:])

            mt4 = S("mt4e", [1, G])
            nc.vector.tensor_reduce(mt4[:],
                                    rmt.rearrange("o (g v) -> o g v", g=G),
                                    AX.X, OP.max)
            s4 = S("s4e", [1, G])
            nc.vector.tensor_reduce(s4[:],
                                    rst.rearrange("o (g v) -> o g v", g=G),
                                    AX.X, OP.add)
            em4 = S("em4e", [1, G])
            nc.scalar.activation(em4[:], mt4[:], AF_EXP, scale=-1.0)
            s4p = S("s4pe", [1, G])
            nc.vector.tensor_tensor(s4p[:], s4[:], em4[:], OP.mult)
            lns4 = S("lns4e", [1, G])
            nc.scalar.activation(lns4[:], s4p[:], AF_LN)
            prob4 = S("prob4e", [1, G])
            nc.vector.reciprocal(prob4[:], s4p[:])
            mtb = (mt4.unsqueeze(2).broadcast_to([1, G, V]))
            eq = S("eqe", [1, 32])
            nc.vector.tensor_tensor(
                eq.rearrange("o (g v) -> o g v", g=G),
                rmt.rearrange("o (g v) -> o g v", g=G), mtb, OP.is_equal)
            cs = S("cse", [1, 32])
            nc.vector.tensor_tensor(cs[:], eq[:], cdt[:], OP.mult)
            cm4 = S("cm4e", [1, G])
            nc.vector.tensor_reduce(cm4[:],
                                    cs.rearrange("o (g v) -> o g v", g=G),
                                    AX.X, OP.max)
            res16 = S("res16e", [1, 4 * G])
            nc.vector.tensor_scalar(res16[:, 0:16:4], cm4[:], -1.0, 8192.0,
                                    OP.mult, OP.add)
            nc.vector.tensor_scalar_mul(res16[:, 1:16:4], lns4[:], -1.0)
            nc.vector.tensor_tensor(res16[:, 2:16:4], prob4[:], lns4[:],
                                    OP.mult)
            nc.vector.memset(res16[:, 3:16:4], 0.0)
            nc.sync.dma_start(
                res_out.ap().rearrange("a b -> (a b)").unsqueeze(0),
                res16[:])

    nc.compile()
    return nc


def _get_program():
    if "nc" not in _PROGRAM_CACHE:
        _PROGRAM_CACHE["nc"] = _build_program()
    return _PROGRAM_CACHE["nc"]


# --------------------------------------------------------------------------
# Host-side sharding / layout prep
# --------------------------------------------------------------------------

def _make_in_maps(inputs):
    gk = np.asarray(inputs["glimpse_K_static"], dtype=np.float32)
    gv = np.asarray(inputs["glimpse_V_static"], dtype=np.float32)
    lk = np.asarray(inputs["logit_K_static"], dtype=np.float32)
    ndf = np.asarray(inputs["node_dynamic_features"], dtype=np.float32)
    vdf = np.asarray(inputs["vehicle_dynamic_features"], dtype=np.float32)
    fc = np.asarray(inputs["fixed_context"], dtype=np.float32)
    msk = np.asarray(inputs["feasibility_mask"])
    w_cs = np.asarray(inputs["W_cs"], dtype=np.float32)
    w_ns = np.asarray(inputs["W_ns"], dtype=np.float32)
    w_out = np.asarray(inputs["W_out"], dtype=np.float32)

    wpackr = np.zeros((128, 16), dtype=np.float32)
    wpackr[:, 0:F_ND] = w_ns[:, D:2 * D].T
    wpackr[:, F_ND:2 * F_ND] = w_ns[:, 2 * D:3 * D].T

    in_maps = []
    for c in range(NCORES):
        bs = slice(c * G, (c + 1) * G)
        kt = np.ascontiguousarray(
            gk[:, bs].transpose(1, 0, 3, 2).reshape(G, 128, N))
        lt = np.ascontiguousarray(lk[bs].transpose(0, 2, 1))
        vn = np.ascontiguousarray(
            gv[:, bs].transpose(1, 2, 0, 3).reshape(G, N, 128))
        nd = ndf[bs]                                   # [G, V, N, F]
        ndftm = np.zeros((G, 72, N), dtype=np.float32)
        ndftm[:, 0:64, :] = nd.transpose(0, 1, 3, 2).reshape(G, 64, N)
        mbx = (msk[bs].astype(np.float32) - 1.0) * MASK_BIG   # [G, V, N]
        ndftm[:, 64:72, :] = mbx
        mbs = np.ascontiguousarray(mbx.transpose(1, 0, 2).reshape(V, G * N))
        ndfn = np.ascontiguousarray(
            nd.reshape(NPAIR, 2, V, N, F_ND)
            .transpose(0, 3, 1, 2, 4).reshape(NPAIR, N, 128))
        wpack = np.zeros((128, 548), dtype=np.float32)
        wpack[:, 0:128] = w_cs[:D]
        wpack[:, 128:256] = w_out
        wpack[0:F_ND, 256:384] = w_ns[:, 0:D]
        wpack[0:F_V, 384:512] = w_cs[D:]
        wpack[:, 512:516] = fc[bs].T
        wpack[0:F_V, 516:548] = vdf[bs].transpose(2, 0, 1).reshape(F_V, 32)
        in_maps.append({
            "kt_in": kt,
            "lt_in": lt,
            "vn_in": vn,
            "ndftm_in": ndftm,
            "mbs_in": mbs,
            "ndfn_in": ndfn,
            "wpack_in": wpack,
            "wpackr_in": wpackr,
        })
    return in_maps


def _postprocess(res_list):
    out = np.concatenate(res_list, axis=0)              # [B, 4]
    a = out[:, 0]
    lp = out[:, 1].astype(np.float32)
    ent = out[:, 2].astype(np.float32)
    sel_vec = (a.astype(np.float32) / np.float32(N)).astype(np.float32)
    sel_node = (np.round(a).astype(np.int64) % N).astype(np.int32)
    return sel_vec, sel_node, lp, ent


LAST_RESULTS = None
ENABLE_LDW_OPT = True
_LDW_PATCHED = False


def _patch_ldw_opt():
    """Flip walrus --enable-ldw-opt (elides redundant PE weight loads)."""
    global _LDW_PATCHED
    if _LDW_PATCHED or not ENABLE_LDW_OPT:
        return
    import concourse.bass_utils as bu
    orig = bu.run_command

    def patched(argv, **kw):
        argv = ["--enable-ldw-opt=true" if a == "--enable-ldw-opt=false"
                else a for a in argv]
        return orig(argv, **kw)

    bu.run_command = patched
    _LDW_PATCHED = True


def _run(inputs, trace=False):
    global LAST_RESULTS
    _patch_ldw_opt()
    from concourse.bass_utils import run_bass_kernel_spmd
    nc = _get_program()
    in_maps = _make_in_maps(inputs)
    res = run_bass_kernel_spmd(nc, in_maps, list(range(NCORES)), trace=trace)
    LAST_RESULTS = res
    return _postprocess([res.results[c]["res_out"] for c in range(NCORES)])


def kernel(**inputs):
    return _run(inputs, trace=False)

